# revision 66
# baseline (speedup 1.0000x reference)
"""BiLinearInteraction Trainium2 kernel (8 NeuronCores, data-parallel over batch).

Reference computation (per pair p=(i,j) of F=26 fields, P=325 pairs):
    out[b, p*64:(p+1)*64] = (x[i, b, :] @ W[p]) * x[j, b, :]
Full shapes: x [26, 4096, 64] f32, W [325, 64, 64] f32 -> out [4096, 20800] f32.

Strategy (measured 103.8us HW, vs 146-156us baseline; HBM roofline ~82us for
the 29.3MB/core of traffic at 358 GB/s/core)
- Shard batch 4096 -> 8 x 512 (4 batch tiles of 128 rows per core); W
  replicated.  Host pre-packs all operands to bf16 so the device is a pure
  stream of matmuls, PSUM drains, elementwise muls and large DMAs.
- Per batch tile, matmuls accumulate into [128, <=2048] PSUM "group" tiles
  (4 banks; 2 pool bufs = whole PSUM).  Groups are classed V/A interleaved:
    V (~21% of cols): DVE multiplies straight out of PSUM (fp32 src, 1x)
      into the bf16 stage tile.
    A: one big ACT copy drains the group PSUM -> stage (bf16, cast), then
      DVE multiplies IN PLACE at 2x (all-bf16 packed mode), per field piece.
  The split balances DVE (1x V muls + 2x A muls ~ 66us) against ACT (1x
  drains ~ 64us); both hide under the HBM-paced ~84us steady phase.
- GPSIMD does NO elementwise work: measured on HW, GpSimd tensor_tensor and
  DVE tensor_tensor serialize on the shared SBUF port pair (the later op
  blocks for the other's full duration), so it adds no throughput.
- PE row groups: the row group of every matmul is its PSUM 512-block's
  global index parity.  Two CONCURRENT different-row-group matmuls writing
  the same PSUM bank are a fatal HW collision (found the hard way; the
  simulator does not model it).  Same-block pieces share a row group
  (serial -> safe) and adjacent blocks alternate, so matmul streams overlap
  in the two 64-row PE halves everywhere, including inside large fields.
  W is packed by block parity (top half partitions 0-63 = even blocks); xt
  carries all fields in both halves (field-parity packings that avoid the
  duplication measured slower: they serialize each field's matmul stream).
- All input loads ride the SP HWDGE ring in first-use order (FIFO makes the
  early chunks absolute priority; spreading loads across rings measured
  20us worse).  First matmul starts ~11us in (after a ~7us fixed preamble).
  Output writes also ride the SP ring: 2 DMAs of ~1.3MB per half batch-tile
  (the last tile's second half uses 4 finer writes to shorten the tail).
- Output staged and written as bf16 (halves the dominant write stream);
  host upcasts to f32.  rel err ~0.0036 (tolerance 2e-2).
"""

import sys

sys.path.insert(0, "/opt/trn_rl_repo")

from itertools import combinations

import ml_dtypes
import numpy as np

import concourse.bass as bass
import concourse.mybir as mybir
from concourse import bacc
from concourse.tile import TileContext

F, D, B = 26, 64, 4096
NCORES = 8
BC = B // NCORES          # 512 batch rows per core
NT = BC // 128            # 4 batch tiles of 128 rows
PAIRS = list(combinations(range(F), 2))
P = len(PAIRS)            # 325
OUT_COLS = P * D          # 20800

N_PAIRS = [F - 1 - i for i in range(F - 1)]           # pairs with left field i
P_START = [sum(N_PAIRS[:i]) for i in range(F - 1)]    # first pair index of field i
FIELD_START = [P_START[i] * D for i in range(F - 1)]  # output col where field i begins
FIELD_END = [FIELD_START[i] + N_PAIRS[i] * D for i in range(F - 1)]

XNW = F * D               # xn cols per batch tile = 1664

# PSUM group grid per batch tile: (c0, c1, class).
# V groups are interleaved between A groups so DVE and ACT stay concurrently
# busy through the tile (a V-first layout measured bulk-synchronous: engines
# took turns in 10-20us phases).
GROUPS = [
    (0, 2048, 'A'), (2048, 4096, 'V'), (4096, 6144, 'A'),
    (6144, 8192, 'A'), (8192, 9856, 'V'),
    (9856, 11904, 'A'), (11904, 13952, 'A'), (13952, 16000, 'A'),
    (16000, 18048, 'A'), (18048, 20096, 'A'), (20096, 20800, 'V'),
]
HALF = 9856               # st0 covers cols [0, 9856), st1 [9856, 20800)
WSPLIT = 2                # write DMAs per half batch-tile

# Contiguous class runs (mul granularity: field pieces within a run).
RUNS = []
for (_g0, _g1, _cls) in GROUPS:
    if RUNS and RUNS[-1][2] == _cls and RUNS[-1][1] == _g0:
        RUNS[-1] = (RUNS[-1][0], _g1, _cls)
    else:
        RUNS.append((_g0, _g1, _cls))

# 512-col PSUM blocks with alternating parity and running per-parity W pack
# offsets (see module docstring for the collision rule this encodes).
BLOCKS = []               # (c0, c1, parity, w_off)
_offs = [0, 0]
_idx = 0
for (_g0, _g1, _cls) in GROUPS:
    _c = _g0
    while _c < _g1:
        _c1 = min(_c + 512, _g1)
        _par = _idx % 2
        BLOCKS.append((_c, _c1, _par, _offs[_par]))
        _offs[_par] += _c1 - _c
        _idx += 1
        _c = _c1
W_COLS = max(_offs)       # top half cols; the shorter half is padded

# W load chunk boundaries (w-offset space).  Finer early chunks let the
# first matmuls start as soon as ~330KB have landed.
W_BOUNDS = [0, 1024, 3072, 6144, W_COLS]
# Output cols where a 512-block straddles a W chunk boundary in w-offset
# space: matmul pieces split there so each rhs lives in one load tile.
W_EXTRA_SPLITS = []
for (_c0, _c1, _par, _boff) in BLOCKS:
    for _s in W_BOUNDS[1:-1]:
        if _boff < _s < _boff + (_c1 - _c0):
            W_EXTRA_SPLITS.append(_c0 + (_s - _boff))

# xt pack: every field in both halves (any block parity can use any field).
XT_OFF = {i: i * 128 for i in range(F)}
XTW = F * 128             # xt cols per batch tile (per half) = 3328

# V-class columns are emitted as fp8 e4m3 into a separate compact output
# (their DVE muls run at 1x regardless of out dtype, and fp8 halves their
# share of the dominant write stream: -2.2MB/core HBM).  Measured rel err
# 0.0126 vs the 2e-2 tolerance.  A-class muls must stay bf16 (the 2x DVE
# mode requires 2-byte operands).
V_OFF = {}                # V group start col -> offset in compact V layout
_vo = 0
for (_g0, _g1, _cls) in GROUPS:
    if _cls == 'V':
        V_OFF[_g0] = _vo
        _vo += _g1 - _g0
V_COLS = _vo              # 4416
A_SEGS = []               # contiguous non-V col ranges (bf16 write segments)
for (_g0, _g1, _cls) in GROUPS:
    if _cls != 'V':
        if A_SEGS and A_SEGS[-1][1] == _g0:
            A_SEGS[-1] = (A_SEGS[-1][0], _g1)
        else:
            A_SEGS.append((_g0, _g1))

F32 = mybir.dt.float32
BF16 = mybir.dt.bfloat16
F8E4 = mybir.dt.float8e4


def _block_of(col):
    for b in BLOCKS:
        if b[0] <= col < b[1]:
            return b
    raise ValueError(col)


def _even_splits(c0, c1, n):
    step = max(-(-((c1 - c0) // n) // 64) * 64, 64)
    out = []
    c = c0
    while c < c1:
        out.append((c, min(c + step, c1)))
        c += step
    return out


def _field_of(col):
    for i in range(F - 1):
        if FIELD_START[i] <= col < FIELD_END[i]:
            return i
    raise ValueError(col)


def _pieces(c0, c1, extra=()):
    """Split [c0, c1) at field starts and any extra boundaries.
    Returns list of (p0, p1, field)."""
    bounds = {c0, c1}
    bounds.update(s for s in FIELD_START if c0 < s < c1)
    bounds.update(e for e in extra if c0 < e < c1)
    bs = sorted(bounds)
    return [(a, b, _field_of(a)) for a, b in zip(bs, bs[1:])]


def _mm_pieces(g0, g1):
    """Matmul pieces: additionally split at 512-col PSUM bank boundaries
    (relative to the group base = block boundaries) and at w-load splits."""
    extra = set(range(g0 + 512, g1, 512))
    extra.update(W_EXTRA_SPLITS)
    return _pieces(g0, g1, extra)


def build_bass() -> bass.Bass:
    # Bacc (not Bass): its compile() splits multi-sem waits into event
    # semaphores -- TRN2 engine instructions take at most one inline wait.
    nc = bacc.Bacc()
    xn = nc.declare_dram_parameter("xn", [128, NT * XNW], BF16, isOutput=False)
    xt = nc.declare_dram_parameter("xt", [128, NT * XTW], BF16, isOutput=False)
    w = nc.declare_dram_parameter("w", [128, W_COLS], BF16, isOutput=False)
    out = nc.declare_dram_parameter("out", [BC, OUT_COLS], BF16, isOutput=True)
    out_v = nc.declare_dram_parameter("out_v", [BC, V_COLS], F8E4, isOutput=True)

    with TileContext(nc) as tc:
        with (
            tc.tile_pool(name="consts", bufs=1) as consts,
            tc.tile_pool(name="stage", bufs=4) as stage_pool,
            tc.tile_pool(name="psum", bufs=2, space="PSUM") as psum_pool,
        ):
            # Separate tile objects per load DMA keep dependency granularity
            # at the piece level.
            w_t = [consts.tile([128, b - a], BF16, tag=f"w{k}", name=f"w{k}")
                   for k, (a, b) in enumerate(zip(W_BOUNDS, W_BOUNDS[1:]))]
            xt00 = consts.tile([128, 256], BF16, tag="xt00", name="xt00")
            xt0r = consts.tile([128, XTW - 256], BF16, tag="xt0r", name="xt0r")
            xtr = consts.tile([128, (NT - 1) * XTW], BF16, tag="xtr", name="xtr")
            xn0 = consts.tile([128, XNW], BF16, tag="xn0", name="xn0")
            xnr = consts.tile([128, (NT - 1) * XNW], BF16, tag="xnr", name="xnr")

            # All loads on the SP ring in first-use order: the FIFO ring
            # drains them in priority order, so the critical early chunks
            # never share bandwidth with the later bulk loads.
            nc.sync.dma_start(out=xt00[:], in_=xt[:, 0:256])
            nc.sync.dma_start(out=w_t[0][:], in_=w[:, W_BOUNDS[0]:W_BOUNDS[1]])
            nc.sync.dma_start(out=xn0[:], in_=xn[:, 0:XNW])
            nc.sync.dma_start(out=xt0r[:], in_=xt[:, 256:XTW])
            nc.sync.dma_start(out=w_t[1][:], in_=w[:, W_BOUNDS[1]:W_BOUNDS[2]])
            nc.sync.dma_start(out=w_t[2][:], in_=w[:, W_BOUNDS[2]:W_BOUNDS[3]])
            nc.sync.dma_start(out=w_t[3][:], in_=w[:, W_BOUNDS[3]:W_BOUNDS[4]])
            nc.sync.dma_start(out=xtr[:], in_=xt[:, XTW:NT * XTW])
            nc.sync.dma_start(out=xnr[:], in_=xn[:, XNW:NT * XNW])

            def xt_slice(t, i, r0):
                c = XT_OFF[i]
                if t == 0:
                    if c + 128 <= 256:
                        return xt00[r0:r0 + D, c:c + 128]
                    return xt0r[r0:r0 + D, c - 256:c - 256 + 128]
                c += (t - 1) * XTW
                return xtr[r0:r0 + D, c:c + 128]

            def w_slice(c0, c1):
                b0, b1, par, boff = _block_of(c0)
                assert c1 <= b1, (c0, c1, b0, b1)
                r0 = par * D
                wc = boff + (c0 - b0)
                n = c1 - c0
                for k in range(len(W_BOUNDS) - 1):
                    if wc + n <= W_BOUNDS[k + 1]:
                        assert wc >= W_BOUNDS[k], (c0, c1, wc)
                        wk = wc - W_BOUNDS[k]
                        return r0, w_t[k][r0:r0 + D, wk:wk + n]
                raise AssertionError((c0, c1, wc))

            def xn_slice(t, i, c0, c1):
                c = (i + 1) * D + (c0 - FIELD_START[i])
                if t > 0:
                    c += (t - 1) * XNW
                src = xn0 if t == 0 else xnr
                return src[:, c:c + (c1 - c0)]

            for t in range(NT):
                st0 = stage_pool.tile([128, HALF], BF16, tag="stage",
                                      name=f"st{t}a")
                st1 = stage_pool.tile([128, OUT_COLS - HALF], BF16, tag="stage",
                                      name=f"st{t}b")
                stv = stage_pool.tile([128, V_COLS], F8E4, tag="stagev",
                                      name=f"st{t}v")

                def st_slice(c0, c1):
                    if c0 >= HALF:
                        return st1[:, c0 - HALF:c1 - HALF]
                    assert c1 <= HALF
                    return st0[:, c0:c1]

                # A-run mul pieces not yet emitted, per run index.
                pending = {}
                for ri, (r0_, r1_, rcls) in enumerate(RUNS):
                    if rcls != 'V':
                        pending[ri] = _pieces(r0_, r1_)

                for (g0, g1, gcls) in GROUPS:
                    ps = psum_pool.tile([128, g1 - g0], F32, tag="ps",
                                        name=f"psx{t}_{g0}")
                    for (c0, c1, i) in _mm_pieces(g0, g1):
                        r0, rhs = w_slice(c0, c1)
                        nc.tensor.matmul(
                            ps[:, c0 - g0:c1 - g0],
                            xt_slice(t, i, r0),
                            rhs,
                            start=True, stop=True,
                        )
                    if gcls == 'V':
                        vo = V_OFF[g0]
                        for (c0, c1, i) in _pieces(g0, g1):
                            nc.vector.tensor_mul(
                                stv[:, vo + (c0 - g0):vo + (c1 - g0)],
                                ps[:, c0 - g0:c1 - g0],
                                xn_slice(t, i, c0, c1),
                            )
                    else:
                        ri = next(k for k, (a, b, cl) in enumerate(RUNS)
                                  if a <= g0 < b)
                        # One big ACT drain (f32 PSUM -> bf16, cast), then
                        # in-place DVE muls for run pieces fully drained now.
                        nc.scalar.copy(out=st_slice(g0, g1), in_=ps[:])
                        done = [pc for pc in pending[ri] if pc[1] <= g1]
                        for (c0, c1, i) in done:
                            pending[ri].remove((c0, c1, i))
                            nc.vector.tensor_mul(
                                st_slice(c0, c1),
                                st_slice(c0, c1),
                                xn_slice(t, i, c0, c1),
                            )
                    if g1 == HALF:
                        # bf16 writes for the A segments inside [0, HALF)
                        for (a, b) in A_SEGS:
                            if b > HALF:
                                continue
                            nc.sync.dma_start(
                                out=out[t * 128:(t + 1) * 128, a:b],
                                in_=st0[:, a:b],
                            )
                assert all(not v for v in pending.values())
                # Last tile: finer writes so the final transfer starts as
                # early as possible (it is the kernel's tail).
                ws = WSPLIT * 2 if t == NT - 1 else WSPLIT
                for (a0, b0) in A_SEGS:
                    if a0 < HALF:
                        continue
                    for (a, b) in _even_splits(a0, b0, ws):
                        nc.sync.dma_start(
                            out=out[t * 128:(t + 1) * 128, a:b],
                            in_=st1[:, a - HALF:b - HALF],
                        )
                nc.sync.dma_start(
                    out=out_v[t * 128:(t + 1) * 128, :], in_=stv[:]
                )
    nc.compile()
    return nc


def prep_inputs(x: np.ndarray, W: np.ndarray):
    """Full inputs -> per-core in_maps with block-parity-packed bf16 layouts."""
    x = np.ascontiguousarray(np.asarray(x, dtype=np.float32))
    W = np.ascontiguousarray(np.asarray(W, dtype=np.float32))
    wg = W.transpose(1, 0, 2).reshape(D, OUT_COLS)
    w_top = np.zeros((D, W_COLS), np.float32)
    w_bot = np.zeros((D, W_COLS), np.float32)
    for (c0, c1, par, boff) in BLOCKS:
        dst = w_top if par == 0 else w_bot
        dst[:, boff:boff + (c1 - c0)] = wg[:, c0:c1]
    w_p = np.ascontiguousarray(
        np.concatenate([w_top, w_bot], axis=0).astype(ml_dtypes.bfloat16)
    )
    in_maps = []
    for c in range(NCORES):
        xc = x[:, c * BC:(c + 1) * BC, :]                       # [26, 512, 64]
        # xn_p[r, t*1664 + f*64 + e] = xc[f, t*128+r, e]
        xn_p = np.ascontiguousarray(
            xc.reshape(F, NT, 128, D).transpose(2, 1, 0, 3)
            .reshape(128, NT * XNW).astype(ml_dtypes.bfloat16)
        )
        # xtd[d, t*XTW + f*128 + r] = xc[f, t*128+r, d], duplicated to both
        # partition halves so any block parity can use any field.
        xtd = (xc.reshape(F, NT, 128, D).transpose(3, 1, 0, 2)
               .reshape(D, NT * XTW))
        xt_p = np.ascontiguousarray(
            np.concatenate([xtd, xtd], axis=0).astype(ml_dtypes.bfloat16)
        )
        in_maps.append({"xn": xn_p, "xt": xt_p, "w": w_p})
    return in_maps


_CACHED_NC = None


def kernel(x: np.ndarray, W: np.ndarray) -> np.ndarray:
    global _CACHED_NC
    from concourse.bass_utils import run_bass_kernel_spmd

    if _CACHED_NC is None:
        _CACHED_NC = build_bass()
    in_maps = prep_inputs(x, W)
    res = run_bass_kernel_spmd(_CACHED_NC, in_maps, list(range(NCORES)))
    shards = []
    for c in range(NCORES):
        full = np.asarray(res.results[c]["out"]).astype(np.float32)
        ov = np.asarray(res.results[c]["out_v"]).astype(np.float32)
        for (g0, g1, cls) in GROUPS:
            if cls == 'V':
                vo = V_OFF[g0]
                full[:, g0:g1] = ov[:, vo:vo + (g1 - g0)]
        shards.append(full)
    return np.concatenate(shards, axis=0)


# revision 67
# speedup vs baseline: 1.0067x; 1.0067x over previous
"""BiLinearInteraction Trainium2 kernel (8 NeuronCores, data-parallel over batch).

Reference computation (per pair p=(i,j) of F=26 fields, P=325 pairs):
    out[b, p*64:(p+1)*64] = (x[i, b, :] @ W[p]) * x[j, b, :]
Full shapes: x [26, 4096, 64] f32, W [325, 64, 64] f32 -> out [4096, 20800] f32.

Strategy (measured 103.8us HW, vs 146-156us baseline; HBM roofline ~82us for
the 29.3MB/core of traffic at 358 GB/s/core)
- Shard batch 4096 -> 8 x 512 (4 batch tiles of 128 rows per core); W
  replicated.  Host pre-packs all operands to bf16 so the device is a pure
  stream of matmuls, PSUM drains, elementwise muls and large DMAs.
- Per batch tile, matmuls accumulate into [128, <=2048] PSUM "group" tiles
  (4 banks; 2 pool bufs = whole PSUM).  Groups are classed V/A interleaved:
    V (~21% of cols): DVE multiplies straight out of PSUM (fp32 src, 1x)
      into the bf16 stage tile.
    A: one big ACT copy drains the group PSUM -> stage (bf16, cast), then
      DVE multiplies IN PLACE at 2x (all-bf16 packed mode), per field piece.
  The split balances DVE (1x V muls + 2x A muls ~ 66us) against ACT (1x
  drains ~ 64us); both hide under the HBM-paced ~84us steady phase.
- GPSIMD does NO elementwise work: measured on HW, GpSimd tensor_tensor and
  DVE tensor_tensor serialize on the shared SBUF port pair (the later op
  blocks for the other's full duration), so it adds no throughput.
- PE row groups: the row group of every matmul is its PSUM 512-block's
  global index parity.  Two CONCURRENT different-row-group matmuls writing
  the same PSUM bank are a fatal HW collision (found the hard way; the
  simulator does not model it).  Same-block pieces share a row group
  (serial -> safe) and adjacent blocks alternate, so matmul streams overlap
  in the two 64-row PE halves everywhere, including inside large fields.
  W is packed by block parity (top half partitions 0-63 = even blocks); xt
  carries all fields in both halves (field-parity packings that avoid the
  duplication measured slower: they serialize each field's matmul stream).
- All input loads ride the SP HWDGE ring in first-use order (FIFO makes the
  early chunks absolute priority; spreading loads across rings measured
  20us worse).  First matmul starts ~11us in (after a ~7us fixed preamble).
  Output writes also ride the SP ring: 2 DMAs of ~1.3MB per half batch-tile
  (the last tile's second half uses 4 finer writes to shorten the tail).
- Output staged and written as bf16 (halves the dominant write stream);
  host upcasts to f32.  rel err ~0.0036 (tolerance 2e-2).
"""

import sys

sys.path.insert(0, "/opt/trn_rl_repo")

from itertools import combinations

import ml_dtypes
import numpy as np

import concourse.bass as bass
import concourse.mybir as mybir
from concourse import bacc
from concourse.tile import TileContext

F, D, B = 26, 64, 4096
NCORES = 8
BC = B // NCORES          # 512 batch rows per core
NT = BC // 128            # 4 batch tiles of 128 rows
PAIRS = list(combinations(range(F), 2))
P = len(PAIRS)            # 325
OUT_COLS = P * D          # 20800

N_PAIRS = [F - 1 - i for i in range(F - 1)]           # pairs with left field i
P_START = [sum(N_PAIRS[:i]) for i in range(F - 1)]    # first pair index of field i
FIELD_START = [P_START[i] * D for i in range(F - 1)]  # output col where field i begins
FIELD_END = [FIELD_START[i] + N_PAIRS[i] * D for i in range(F - 1)]

XNW = F * D               # xn cols per batch tile = 1664

# PSUM group grid per batch tile: (c0, c1, class).
# V groups are interleaved between A groups so DVE and ACT stay concurrently
# busy through the tile (a V-first layout measured bulk-synchronous: engines
# took turns in 10-20us phases).
GROUPS = [
    (0, 2048, 'A'), (2048, 4096, 'V'), (4096, 6144, 'A'),
    (6144, 8192, 'A'), (8192, 9856, 'V'),
    (9856, 11904, 'A'), (11904, 13952, 'A'), (13952, 16000, 'A'),
    (16000, 18048, 'A'), (18048, 20096, 'A'), (20096, 20800, 'V'),
]
HALF = 9856               # st0 covers cols [0, 9856), st1 [9856, 20800)
WSPLIT = 2                # write DMAs per half batch-tile

# Contiguous class runs (mul granularity: field pieces within a run).
RUNS = []
for (_g0, _g1, _cls) in GROUPS:
    if RUNS and RUNS[-1][2] == _cls and RUNS[-1][1] == _g0:
        RUNS[-1] = (RUNS[-1][0], _g1, _cls)
    else:
        RUNS.append((_g0, _g1, _cls))

# 512-col PSUM blocks with alternating parity and running per-parity W pack
# offsets (see module docstring for the collision rule this encodes).
BLOCKS = []               # (c0, c1, parity, w_off)
_offs = [0, 0]
_idx = 0
for (_g0, _g1, _cls) in GROUPS:
    _c = _g0
    while _c < _g1:
        _c1 = min(_c + 512, _g1)
        _par = _idx % 2
        BLOCKS.append((_c, _c1, _par, _offs[_par]))
        _offs[_par] += _c1 - _c
        _idx += 1
        _c = _c1
W_COLS = max(_offs)       # top half cols; the shorter half is padded

# W load chunk boundaries (w-offset space).  Finer early chunks let the
# first matmuls start as soon as ~330KB have landed.
W_BOUNDS = [0, 1024, 3072, 6144, W_COLS]
# Output cols where a 512-block straddles a W chunk boundary in w-offset
# space: matmul pieces split there so each rhs lives in one load tile.
W_EXTRA_SPLITS = []
for (_c0, _c1, _par, _boff) in BLOCKS:
    for _s in W_BOUNDS[1:-1]:
        if _boff < _s < _boff + (_c1 - _c0):
            W_EXTRA_SPLITS.append(_c0 + (_s - _boff))

# xt pack: every field in both halves (any block parity can use any field).
XT_OFF = {i: i * 128 for i in range(F)}
XTW = F * 128             # xt cols per batch tile (per half) = 3328

F32 = mybir.dt.float32
BF16 = mybir.dt.bfloat16


def _block_of(col):
    for b in BLOCKS:
        if b[0] <= col < b[1]:
            return b
    raise ValueError(col)


def _even_splits(c0, c1, n):
    step = max(-(-((c1 - c0) // n) // 64) * 64, 64)
    out = []
    c = c0
    while c < c1:
        out.append((c, min(c + step, c1)))
        c += step
    return out


def _field_of(col):
    for i in range(F - 1):
        if FIELD_START[i] <= col < FIELD_END[i]:
            return i
    raise ValueError(col)


def _pieces(c0, c1, extra=()):
    """Split [c0, c1) at field starts and any extra boundaries.
    Returns list of (p0, p1, field)."""
    bounds = {c0, c1}
    bounds.update(s for s in FIELD_START if c0 < s < c1)
    bounds.update(e for e in extra if c0 < e < c1)
    bs = sorted(bounds)
    return [(a, b, _field_of(a)) for a, b in zip(bs, bs[1:])]


def _mm_pieces(g0, g1):
    """Matmul pieces: additionally split at 512-col PSUM bank boundaries
    (relative to the group base = block boundaries) and at w-load splits."""
    extra = set(range(g0 + 512, g1, 512))
    extra.update(W_EXTRA_SPLITS)
    return _pieces(g0, g1, extra)


def build_bass() -> bass.Bass:
    # Bacc (not Bass): its compile() splits multi-sem waits into event
    # semaphores -- TRN2 engine instructions take at most one inline wait.
    nc = bacc.Bacc()
    xn = nc.declare_dram_parameter("xn", [128, NT * XNW], BF16, isOutput=False)
    xt = nc.declare_dram_parameter("xt", [128, NT * XTW], BF16, isOutput=False)
    w = nc.declare_dram_parameter("w", [128, W_COLS], BF16, isOutput=False)
    out = nc.declare_dram_parameter("out", [BC, OUT_COLS], BF16, isOutput=True)

    with TileContext(nc) as tc:
        with (
            tc.tile_pool(name="consts", bufs=1) as consts,
            tc.tile_pool(name="stage", bufs=4) as stage_pool,
            tc.tile_pool(name="psum", bufs=2, space="PSUM") as psum_pool,
        ):
            # Separate tile objects per load DMA keep dependency granularity
            # at the piece level.
            w_t = [consts.tile([128, b - a], BF16, tag=f"w{k}", name=f"w{k}")
                   for k, (a, b) in enumerate(zip(W_BOUNDS, W_BOUNDS[1:]))]
            xt00 = consts.tile([128, 256], BF16, tag="xt00", name="xt00")
            xt0r = consts.tile([128, XTW - 256], BF16, tag="xt0r", name="xt0r")
            xtr = consts.tile([128, (NT - 1) * XTW], BF16, tag="xtr", name="xtr")
            xn0 = consts.tile([128, XNW], BF16, tag="xn0", name="xn0")
            xnr = consts.tile([128, (NT - 1) * XNW], BF16, tag="xnr", name="xnr")

            # All loads on the SP ring in first-use order: the FIFO ring
            # drains them in priority order, so the critical early chunks
            # never share bandwidth with the later bulk loads.
            nc.sync.dma_start(out=xt00[:], in_=xt[:, 0:256])
            nc.sync.dma_start(out=w_t[0][:], in_=w[:, W_BOUNDS[0]:W_BOUNDS[1]])
            nc.sync.dma_start(out=xn0[:], in_=xn[:, 0:XNW])
            nc.sync.dma_start(out=xt0r[:], in_=xt[:, 256:XTW])
            nc.sync.dma_start(out=w_t[1][:], in_=w[:, W_BOUNDS[1]:W_BOUNDS[2]])
            nc.sync.dma_start(out=w_t[2][:], in_=w[:, W_BOUNDS[2]:W_BOUNDS[3]])
            nc.sync.dma_start(out=w_t[3][:], in_=w[:, W_BOUNDS[3]:W_BOUNDS[4]])
            nc.sync.dma_start(out=xtr[:], in_=xt[:, XTW:NT * XTW])
            nc.sync.dma_start(out=xnr[:], in_=xn[:, XNW:NT * XNW])

            def xt_slice(t, i, r0):
                c = XT_OFF[i]
                if t == 0:
                    if c + 128 <= 256:
                        return xt00[r0:r0 + D, c:c + 128]
                    return xt0r[r0:r0 + D, c - 256:c - 256 + 128]
                c += (t - 1) * XTW
                return xtr[r0:r0 + D, c:c + 128]

            def w_slice(c0, c1):
                b0, b1, par, boff = _block_of(c0)
                assert c1 <= b1, (c0, c1, b0, b1)
                r0 = par * D
                wc = boff + (c0 - b0)
                n = c1 - c0
                for k in range(len(W_BOUNDS) - 1):
                    if wc + n <= W_BOUNDS[k + 1]:
                        assert wc >= W_BOUNDS[k], (c0, c1, wc)
                        wk = wc - W_BOUNDS[k]
                        return r0, w_t[k][r0:r0 + D, wk:wk + n]
                raise AssertionError((c0, c1, wc))

            def xn_slice(t, i, c0, c1):
                c = (i + 1) * D + (c0 - FIELD_START[i])
                if t > 0:
                    c += (t - 1) * XNW
                src = xn0 if t == 0 else xnr
                return src[:, c:c + (c1 - c0)]

            for t in range(NT):
                st0 = stage_pool.tile([128, HALF], BF16, tag="stage",
                                      name=f"st{t}a")
                st1 = stage_pool.tile([128, OUT_COLS - HALF], BF16, tag="stage",
                                      name=f"st{t}b")

                def st_slice(c0, c1):
                    if c0 >= HALF:
                        return st1[:, c0 - HALF:c1 - HALF]
                    assert c1 <= HALF
                    return st0[:, c0:c1]

                # A-run mul pieces not yet emitted, per run index.
                pending = {}
                for ri, (r0_, r1_, rcls) in enumerate(RUNS):
                    if rcls != 'V':
                        pending[ri] = _pieces(r0_, r1_)

                for (g0, g1, gcls) in GROUPS:
                    ps = psum_pool.tile([128, g1 - g0], F32, tag="ps",
                                        name=f"psx{t}_{g0}")
                    for (c0, c1, i) in _mm_pieces(g0, g1):
                        r0, rhs = w_slice(c0, c1)
                        nc.tensor.matmul(
                            ps[:, c0 - g0:c1 - g0],
                            xt_slice(t, i, r0),
                            rhs,
                            start=True, stop=True,
                        )
                    if gcls == 'V':
                        for (c0, c1, i) in _pieces(g0, g1):
                            nc.vector.tensor_mul(
                                st_slice(c0, c1),
                                ps[:, c0 - g0:c1 - g0],
                                xn_slice(t, i, c0, c1),
                            )
                    else:
                        ri = next(k for k, (a, b, cl) in enumerate(RUNS)
                                  if a <= g0 < b)
                        # One big ACT drain (f32 PSUM -> bf16, cast), then
                        # in-place DVE muls for run pieces fully drained now.
                        nc.scalar.copy(out=st_slice(g0, g1), in_=ps[:])
                        done = [pc for pc in pending[ri] if pc[1] <= g1]
                        for (c0, c1, i) in done:
                            pending[ri].remove((c0, c1, i))
                            nc.vector.tensor_mul(
                                st_slice(c0, c1),
                                st_slice(c0, c1),
                                xn_slice(t, i, c0, c1),
                            )
                    if g1 == HALF:
                        for (a, b) in _even_splits(0, HALF, WSPLIT):
                            nc.sync.dma_start(
                                out=out[t * 128:(t + 1) * 128, a:b],
                                in_=st0[:, a:b],
                            )
                assert all(not v for v in pending.values())
                # Last tile: finer writes so the final transfer starts as
                # early as possible (it is the kernel's tail).
                ws = WSPLIT * 2 if t == NT - 1 else WSPLIT
                for (a, b) in _even_splits(HALF, OUT_COLS, ws):
                    nc.sync.dma_start(
                        out=out[t * 128:(t + 1) * 128, a:b],
                        in_=st1[:, a - HALF:b - HALF],
                    )
    nc.compile()
    return nc


def prep_inputs(x: np.ndarray, W: np.ndarray):
    """Full inputs -> per-core in_maps with block-parity-packed bf16 layouts."""
    x = np.ascontiguousarray(np.asarray(x, dtype=np.float32))
    W = np.ascontiguousarray(np.asarray(W, dtype=np.float32))
    wg = W.transpose(1, 0, 2).reshape(D, OUT_COLS)
    w_top = np.zeros((D, W_COLS), np.float32)
    w_bot = np.zeros((D, W_COLS), np.float32)
    for (c0, c1, par, boff) in BLOCKS:
        dst = w_top if par == 0 else w_bot
        dst[:, boff:boff + (c1 - c0)] = wg[:, c0:c1]
    w_p = np.ascontiguousarray(
        np.concatenate([w_top, w_bot], axis=0).astype(ml_dtypes.bfloat16)
    )
    in_maps = []
    for c in range(NCORES):
        xc = x[:, c * BC:(c + 1) * BC, :]                       # [26, 512, 64]
        # xn_p[r, t*1664 + f*64 + e] = xc[f, t*128+r, e]
        xn_p = np.ascontiguousarray(
            xc.reshape(F, NT, 128, D).transpose(2, 1, 0, 3)
            .reshape(128, NT * XNW).astype(ml_dtypes.bfloat16)
        )
        # xtd[d, t*XTW + f*128 + r] = xc[f, t*128+r, d], duplicated to both
        # partition halves so any block parity can use any field.
        xtd = (xc.reshape(F, NT, 128, D).transpose(3, 1, 0, 2)
               .reshape(D, NT * XTW))
        xt_p = np.ascontiguousarray(
            np.concatenate([xtd, xtd], axis=0).astype(ml_dtypes.bfloat16)
        )
        in_maps.append({"xn": xn_p, "xt": xt_p, "w": w_p})
    return in_maps


_CACHED_NC = None


def kernel(x: np.ndarray, W: np.ndarray) -> np.ndarray:
    global _CACHED_NC
    from concourse.bass_utils import run_bass_kernel_spmd

    if _CACHED_NC is None:
        _CACHED_NC = build_bass()
    in_maps = prep_inputs(x, W)
    res = run_bass_kernel_spmd(_CACHED_NC, in_maps, list(range(NCORES)))
    shards = [
        np.asarray(res.results[c]["out"]).astype(np.float32) for c in range(NCORES)
    ]
    return np.concatenate(shards, axis=0)
